# revision 26
# baseline (speedup 1.0000x reference)
"""Trainium2 Bass kernel v3 for the dense transformer block (cross-attn + FFN).

Problem: nn_MAB (B=4, nq=nk=1024, D=1024, H=16, HD=64), fp32 in/out.

Sharding: data-parallel, zero collectives. 8 cores = 4 batches x 2
query-halves; each core computes 512 query rows of one batch end-to-end.

v3 changes vs v2 (bf16 baseline, ~146us):
  - fp8(e4m3) attention path: Y/X/Wq/Wk/Wv/Wo quantized to TRN e4m3
    (clip +-240).  Weights host-scaled x32 (x16 for Wo) to stay clear of
    fp8 subnormals; the compensating 1/8192 folds into the exp activation
    scale and 1/512 into the O-proj eviction.  Numpy end-to-end emulation:
    rel err 4.1e-3 vs 3.9e-3 all-bf16 (tol 2e-2) -- attention output is
    ~3% of the residual stream, so its quantization noise is negligible.
  - DoubleRow perf mode (2 fp8 MACs/cell/cycle) for the contraction-1024
    matmuls: V/K/Q projections and the O projection.  Scores (K=64) and
    attnV (M=65 ones-column denominator trick) stay normal-mode fp8
    (1 cyc/row, same as bf16).
  - X residual path stays bf16 (fp8 there would put 3.6% straight into
    the output); FFN stays bf16.
  - yt layout changed to [P, NJ(feat-block), nt(keys)] so the same SBUF
    tile serves V-proj DR lhsT ([128,2,128] feat-pair slices) and K-proj
    DR rhs ([128,2,<=256] key chunks).

Measured v2: 145779ns harness; v3 target ~120us (PE busy 133->~110us).
"""

import numpy as np
import ml_dtypes

import concourse.bass as bass
import concourse.mybir as mybir
import concourse.tile as tile
from concourse import bacc
from concourse.bass_utils import run_bass_kernel_spmd

F32 = mybir.dt.float32
F32R = mybir.dt.float32r
BF16 = mybir.dt.bfloat16
F8 = mybir.dt.float8e4
AF = mybir.ActivationFunctionType
DR = mybir.MatmulPerfMode.DoubleRow

D = 1024          # model dim
P = 128           # partitions
NJ = D // P       # feature tiles (8)
NP = NJ // 2      # feature-pair tiles for DoubleRow (4)
NQ = 512          # queries per core
H = 16
HD = 64
NPAIR = H // 2    # head pairs (8)
NEG = -30000.0    # additive mask for dropped/pad keys
EPS = 1e-5
QC = 256          # tail query-chunk
NQC = NQ // QC    # tail chunks (2)
WS = 32.0         # fp8 weight scale (wq/wk/wv); wo uses WS/2
ESC = 1.0 / (WS * WS * 8.0)   # exp activation scale (1/8192)
OSC = 1.0 / (WS * WS / 2.0)   # o-proj eviction scale (1/512)
F1S = 8.0         # ff1 relu eviction scale (fp8; b1 host-premultiplied)
W2S = 4.0         # W2 fp8 host scale -> ffn2 psum = 32*ff


def build_nc(nkt: int, reps: int = 1, nkp: int | None = None) -> bass.Bass:
    """nkt = number of 128-key tiles (5 for the compacted fast path).
    nkp = key count the K-projection actually computes (multiple of 32,
    >= max valid keys; the kt pad region [nkp:nt] is zeroed so masked
    scores stay finite). reps > 1 wraps the body in a hardware loop
    (timing amplification only)."""
    from contextlib import nullcontext
    nt = nkt * P
    if nkp is None:
        nkp = nt
    assert nkp % 32 == 0 and 0 < nkp <= nt
    nc = bacc.Bacc("TRN2", target_bir_lowering=False, debug=False)

    # ---- DRAM I/O (per-core shards; host prepares exact SBUF layouts) ----
    xt = nc.dram_tensor("xt", [P, NJ * NQ], BF16, kind="ExternalInput")[:]
    xt8 = nc.dram_tensor("xt8", [P, NJ * NQ], F8, kind="ExternalInput")[:]
    yt = nc.dram_tensor("yt", [P, NJ * nt], F8, kind="ExternalInput")[:]
    wq = nc.dram_tensor("wq", [P, NJ * D], F8, kind="ExternalInput")[:]
    wk = nc.dram_tensor("wk", [P, NJ * D], F8, kind="ExternalInput")[:]
    wv = nc.dram_tensor("wv", [P, NJ * D], F8, kind="ExternalInput")[:]
    wo = nc.dram_tensor("wo", [P, NJ * D], F8, kind="ExternalInput")[:]
    w1 = nc.dram_tensor("w1", [P, NJ * D], BF16, kind="ExternalInput")[:]
    w2 = nc.dram_tensor("w2", [P, NJ * D], F8, kind="ExternalInput")[:]
    # one [p, nkt + 7*NJ] tensor: mask bias, then g1/bb1/g2/bb2/b1/b2/bb1b2
    vecs = nc.dram_tensor("vecs", [P, nkt + 7 * NJ], F32, kind="ExternalInput")[:]
    outt = nc.dram_tensor("outt", [P, NJ * NQ], F32, kind="ExternalOutput")[:]

    with tile.TileContext(nc) as tc, \
         nc.allow_low_precision(reason="fp8 attention path; tol 2e-2"), \
         (tc.For_i(0, reps) if reps > 1 else nullcontext()), \
         tc.tile_pool(name="persist", bufs=1) as persist, \
         tc.tile_pool(name="psum", bufs=8, space="PSUM") as pp:
        if True:

            def ps_tile(name):
                return pp.tile([P, NQ], F32, tag="ps", name=name)

            yt_sb = persist.tile([P, NJ, nt], F8)
            wv_sb = persist.tile([P, 4, NJ, 256], F8)  # [ci, k, m%256] quarters
            wk_sb = persist.tile([P, NJ, D], F8)
            wq_sb = persist.tile([P, NJ, D], F8)
            wo_sb = persist.tile([P, NJ, D], F8)
            w1_sb = persist.tile([P, NJ, D], BF16)
            w2_sb = persist.tile([P, NJ, D], F8)
            xt_sb = persist.tile([P, NJ, NQ], BF16)
            xt8_sb = persist.tile([P, NJ, NQ], F8)

            # ---- small vectors first (mask_sb gates the first exp) ----
            vec_sb = persist.tile([P, nkt + 7 * NJ], F32)
            nc.sync.dma_start(vec_sb, vecs)
            mask_sb = vec_sb[:, 0:nkt]
            g1_sb = vec_sb[:, nkt + 0 * NJ:nkt + 1 * NJ]
            bb1_sb = vec_sb[:, nkt + 1 * NJ:nkt + 2 * NJ]
            g2_sb = vec_sb[:, nkt + 2 * NJ:nkt + 3 * NJ]
            bb2_sb = vec_sb[:, nkt + 3 * NJ:nkt + 4 * NJ]
            b1_sb = vec_sb[:, nkt + 4 * NJ:nkt + 5 * NJ]
            b2_sb = vec_sb[:, nkt + 5 * NJ:nkt + 6 * NJ]
            bb1b2_sb = vec_sb[:, nkt + 6 * NJ:nkt + 7 * NJ]  # ln1_b + b2

            # big loads, all on the SP queue in first-use order, each one
            # fully contiguous (strided DMAs pay ~2us of per-row descriptor
            # overhead).  wv is host-arranged in [ci, k, m%256] quarters so
            # the first V matmul waits on yt + a quarter of Wv only.
            nc.sync.dma_start(yt_sb, yt.rearrange("p (k m) -> p k m", k=NJ))
            wv_r = wv.rearrange("p (c k m) -> p c k m", c=4, k=NJ)
            for ci in range(4):
                nc.sync.dma_start(wv_sb[:, ci], wv_r[:, ci])
            nc.sync.dma_start(wk_sb, wk.rearrange("p (k m) -> p k m", k=NJ))
            nc.sync.dma_start(xt8_sb, xt8.rearrange("p (j q) -> p j q", j=NJ))
            nc.sync.dma_start(wq_sb, wq.rearrange("p (k m) -> p k m", k=NJ))
            nc.sync.dma_start(wo_sb, wo.rearrange("p (k m) -> p k m", k=NJ))
            nc.sync.dma_start(xt_sb, xt.rearrange("p (j q) -> p j q", j=NJ))
            nc.sync.dma_start(w1_sb, w1.rearrange("p (k m) -> p k m", k=NJ))
            nc.sync.dma_start(w2_sb, w2.rearrange("p (k m) -> p k m", k=NJ))

            # ---- constants ----
            lnw = persist.tile([P, P], BF16)         # 1/D for LN stat matmuls
            nc.vector.memset(lnw, 1.0 / D)
            eps_sb = persist.tile([P, 1], F32)
            nc.vector.memset(eps_sb, EPS)

            # attention head outputs (x32 scale), feature-major; tile j rows
            # 0:64 = head 2j+1, rows 64:128 = head 2j (host permutes Wo rows).
            outT = persist.tile([P, NJ, NQ], F8)

            with tc.tile_pool(name="attn", bufs=1) as big:
                # V (x32) with an interleaved ones column per head
                v_sb = big.tile([P, nkt, H, HD + 1], F8)
                nc.vector.memset(v_sb[:, :, :, HD:HD + 1], 1.0)

                # ---- V = Y @ Wv.T x32 (DoubleRow over feature pairs),
                # quarter-chunks so the first matmul only waits for the
                # yt pairs + a quarter of Wv ----
                for ci in range(4):
                    for t in range(nkt):
                        ps = ps_tile("ps_v")
                        for kp in range(NP):
                            nc.tensor.matmul(
                                ps[:, 0:256],
                                yt_sb[:, 2 * kp:2 * kp + 2, t * P:(t + 1) * P],
                                wv_sb[:, ci, 2 * kp:2 * kp + 2, :],
                                start=(kp == 0), stop=(kp == NP - 1),
                                perf_mode=DR,
                            )
                        nc.vector.tensor_copy(
                            v_sb[:, t, ci * 4:(ci + 1) * 4, 0:HD], ps[:, 0:256])

                # ---- per head-pair: KT, QT, scoresT, exp, attnV ----
                with tc.tile_pool(name="qk", bufs=2) as qkp, \
                     tc.tile_pool(name="exp", bufs=4) as ep, \
                     tc.tile_pool(name="stage", bufs=3) as stp:
                    deferred = []   # (ps_e, ps_o, rc_e, rc_o, j) of prev pair

                    def flush_deferred():
                        for (pse, pso, rce, rco, jj) in deferred:
                            # replicate the reciprocal rows across 64
                            # partitions on the (idle) Pool engine.  HW ucode
                            # broadcasts the physical partition 0, so first
                            # hop the row from partition 64 to 0 via a tiny
                            # shift DMA.
                            rc0o = stp.tile([P, NQ], F32R, tag="rc0", name="rc0o")
                            nc.sync.dma_start(rc0o[0:1, :], rco[HD:HD + 1, :])
                            rc0e = stp.tile([P, NQ], F32R, tag="rc0", name="rc0e")
                            nc.sync.dma_start(rc0e[0:1, :], rce[HD:HD + 1, :])
                            rcb_o = stp.tile([P, NQ], F32R, tag="rcb", name="rcb_o")
                            nc.gpsimd.partition_broadcast(
                                rcb_o[0:HD, :], rc0o[0:1, :])
                            rcb_e = stp.tile([P, NQ], F32R, tag="rcb", name="rcb_e")
                            nc.gpsimd.partition_broadcast(
                                rcb_e[0:HD, :], rc0e[0:1, :])
                            # odd head -> outT rows 0:64 directly (x32 kept)
                            nc.vector.tensor_mul(
                                outT[0:HD, jj, :], pso[0:HD, :], rcb_o[0:HD, :])
                            # even head -> staging, partition-shift to 64:128
                            tmp = stp.tile([P, NQ], F8, tag="tmp", name="tmp")
                            nc.vector.tensor_mul(
                                tmp[0:HD, :], pse[0:HD, :], rcb_e[0:HD, :])
                            nc.sync.dma_start(outT[HD:P, jj, :], tmp[0:HD, :])
                        deferred.clear()

                    def emit_kq(j):
                        """K^T + Q^T matmuls for pair j -> (kt_j, qt_j), both
                        x32-scaled fp8.  Streams only nkp keys; pad region
                        zeroed on the first ring pass."""
                        ms = slice(j * P, (j + 1) * P)
                        kt_j = qkp.tile([P, nt], F8, tag="kt", name="kt_j")
                        if nkp < nt:
                            nc.vector.memset(kt_j[:, nkp:nt], 0.0)
                        # DoubleRow chunks of 256 keys, then a <=224 tail
                        # (normal mode: DR loses below 128 free).
                        chunks = []
                        k0 = 0
                        while k0 < nkp:
                            kn = min(256, nkp - k0)
                            chunks.append((k0, kn))
                            k0 += kn
                        for (k0, kn) in chunks:
                            ks = slice(k0, k0 + kn)
                            ps = ps_tile("ps_k")
                            if kn >= P:
                                for kp in range(NP):
                                    nc.tensor.matmul(
                                        ps[:, 0:kn],
                                        wk_sb[:, 2 * kp:2 * kp + 2, ms],
                                        yt_sb[:, 2 * kp:2 * kp + 2, ks],
                                        start=(kp == 0), stop=(kp == NP - 1),
                                        perf_mode=DR,
                                    )
                            else:
                                for k in range(NJ):
                                    nc.tensor.matmul(
                                        ps[:, 0:kn], wk_sb[:, k, ms],
                                        yt_sb[:, k, ks],
                                        start=(k == 0), stop=(k == NJ - 1),
                                    )
                            nc.vector.tensor_copy(kt_j[:, ks], ps[:, 0:kn])
                        # Q^T m-tile j (x32; /8 folded into exp scale)
                        qt_j = qkp.tile([P, NQ], F8, tag="qt", name="qt_j")
                        for c in range(2):
                            qs = slice(c * 256, (c + 1) * 256)
                            ps = ps_tile("ps_q")
                            for kp in range(NP):
                                nc.tensor.matmul(
                                    ps[:, 0:256],
                                    wq_sb[:, 2 * kp:2 * kp + 2, ms],
                                    xt8_sb[:, 2 * kp:2 * kp + 2, qs],
                                    start=(kp == 0), stop=(kp == NP - 1),
                                    perf_mode=DR,
                                )
                            nc.vector.tensor_copy(qt_j[:, qs], ps[:, 0:256])
                        return kt_j, qt_j

                    def emit_scores(j, kt_j, qt_j):
                        """scoresT + exp for pair j, row-packed on the PE.
                        psum = 8192*s; exp applies scale=1/8192 + mask bias."""
                        exp_e = ep.tile([P, nkt, NQ], F8, tag="exp", name="exp_e")
                        exp_o = ep.tile([P, nkt, NQ], F8, tag="exp", name="exp_o")
                        for t in range(nkt):
                            ks = slice(t * P, (t + 1) * P)
                            ps0 = ps_tile("ps_s0")
                            nc.tensor.matmul(
                                ps0, kt_j[0:HD, ks], qt_j[0:HD, :],
                                start=True, stop=True, tile_position=(0, 0),
                            )
                            ps1 = ps_tile("ps_s1")
                            nc.tensor.matmul(
                                ps1, kt_j[HD:P, ks], qt_j[HD:P, :],
                                start=True, stop=True, tile_position=(HD, 0),
                            )
                            nc.scalar.activation(
                                exp_e[:, t, :], ps0, AF.Exp,
                                bias=mask_sb[:, t:t + 1], scale=ESC)
                            nc.scalar.activation(
                                exp_o[:, t, :], ps1, AF.Exp,
                                bias=mask_sb[:, t:t + 1], scale=ESC)
                        return exp_e, exp_o

                    def emit_attnv(j, exp_e, exp_o):
                        """attnV: lhsT = [V_h*32 | ones] (M=65) -> rows 0:64
                        x32 numerator, row 64 = softmax denominator."""
                        ps_e = ps_tile("ps_ae")
                        ps_o = ps_tile("ps_ao")
                        for t in range(nkt):
                            st, sp = t == 0, t == nkt - 1
                            nc.tensor.matmul(
                                ps_e[0:HD + 1, :], v_sb[:, t, 2 * j, :],
                                exp_e[:, t, :], start=st, stop=sp,
                            )
                        for t in range(nkt):
                            st, sp = t == 0, t == nkt - 1
                            nc.tensor.matmul(
                                ps_o[0:HD + 1, :], v_sb[:, t, 2 * j + 1, :],
                                exp_o[:, t, :], start=st, stop=sp,
                            )
                        # reciprocals (partition 64, f32r) on DVE now; divide
                        # + broadcast run one pair later via flush_deferred
                        rc_e = stp.tile([P, NQ], F32R, tag="rc", name="rc_e")
                        nc.vector.reciprocal(
                            rc_e[HD:HD + 1, :], ps_e[HD:HD + 1, :])
                        rc_o = stp.tile([P, NQ], F32R, tag="rc", name="rc_o")
                        nc.vector.reciprocal(
                            rc_o[HD:HD + 1, :], ps_o[HD:HD + 1, :])
                        deferred.append((ps_e, ps_o, rc_e, rc_o, j))

                    # software pipeline: next pair's K/Q matmuls are issued
                    # between this pair's scores and attnV so PE never waits
                    # on the DVE evictions or the exp activations.
                    kq = emit_kq(0)
                    for j in range(NPAIR):
                        flush_deferred()
                        exps = emit_scores(j, *kq)
                        if j + 1 < NPAIR:
                            kq = emit_kq(j + 1)
                        emit_attnv(j, *exps)
                    flush_deferred()

            # ---- tail: O-proj + LN1 + FFN + LN2, 2 query-chunks ----
            with tc.tile_pool(name="tail", bufs=1) as tl, \
                 tc.tile_pool(name="ln", bufs=8) as lnp:

                x1 = tl.tile([P, NJ, NQ], BF16)
                xsq = tl.tile([P, NJ, NQ], BF16)
                hT = tl.tile([P, NJ, NQ], BF16)
                hTb2 = tl.tile([P, NJ, NQ], BF16)
                ff1 = tl.tile([P, NJ, NQ], F8)
                x2 = tl.tile([P, NJ, NQ], BF16)
                o_sb = tl.tile([P, NQ, NJ], F32)   # query-major for the store

                def oproj(c):
                    """Z = outT.T @ Wo.T via DoubleRow over feature pairs;
                    psum = 512*Z, evicted with a 1/512 scale + X residual."""
                    qs = slice(c * QC, (c + 1) * QC)
                    for m in range(NJ):
                        ps = ps_tile("ps_z")
                        for gp in range(NP):
                            nc.tensor.matmul(
                                ps[:, 0:QC],
                                wo_sb[:, 2 * gp:2 * gp + 2, m * P:(m + 1) * P],
                                outT[:, 2 * gp:2 * gp + 2, qs],
                                start=(gp == 0), stop=(gp == NP - 1),
                                perf_mode=DR,
                            )
                        nc.vector.scalar_tensor_tensor(
                            x1[:, m, qs], ps[:, 0:QC], OSC, xt_sb[:, m, qs],
                            op0=mybir.AluOpType.mult, op1=mybir.AluOpType.add)
                        # eager square so LN1 stats don't wait on ACT later
                        nc.scalar.activation(
                            xsq[:, m, qs], x1[:, m, qs], AF.Square)

                def ln_stats(x_sb, qs, name):
                    """mean/E[x^2] matmuls for query-slice qs (squares
                    already in xsq, computed eagerly at eviction)."""
                    qn = qs.stop - qs.start
                    ps_m = ps_tile(name + "_m")
                    for jj in range(NJ):
                        nc.tensor.matmul(
                            ps_m[:, 0:qn], lnw, x_sb[:, jj, qs],
                            start=(jj == 0), stop=(jj == NJ - 1))
                    ps_v = ps_tile(name + "_v")
                    for jj in range(NJ):
                        nc.tensor.matmul(
                            ps_v[:, 0:qn], lnw, xsq[:, jj, qs],
                            start=(jj == 0), stop=(jj == NJ - 1))
                    return ps_m, ps_v

                # output is query-major [p, q, j] so each wave's store is one
                # fully-contiguous DMA (strided stores pay the ~2.3us
                # 128-descriptor floor per call)
                outt_r = outt.rearrange("p (q j) -> p q j", j=NJ)

                def ln_norm(x_sb, ps_m, ps_v, qs, gv, bv, dest, store=False,
                            dest2=None, bv2=None):
                    """DVE/Pool chain: normalize query-slice qs into dest.
                    Stats psums are read in place (no mean eviction); rstd
                    comes from one ACT Rsqrt.  dest2/bv2 emit a second
                    biased copy (hT + b2 for the ffn2 eviction) on Pool."""
                    qn = qs.stop - qs.start
                    mean = lnp.tile([P, QC], F32, tag="lnt", name="mean")[:, 0:qn]
                    nc.vector.tensor_copy(mean, ps_m[:, 0:qn])
                    var = lnp.tile([P, QC], F32, tag="lnt", name="var")[:, 0:qn]
                    nc.vector.tensor_mul(var, mean, mean)
                    nc.vector.tensor_tensor(
                        var, ps_v[:, 0:qn], var, mybir.AluOpType.subtract)
                    sd = lnp.tile([P, QC], F32, tag="lnt", name="sd")[:, 0:qn]
                    nc.scalar.activation(sd, var, AF.Sqrt, bias=eps_sb, scale=1.0)
                    rstd = lnp.tile([P, QC], BF16, tag="lnr", name="rstd")[:, 0:qn]
                    nc.vector.reciprocal(rstd, sd)
                    mrs = lnp.tile([P, QC], BF16, tag="lnr", name="mrs")[:, 0:qn]
                    nc.vector.tensor_mul(mrs, mean, rstd)
                    for jj in range(NJ):
                        t = lnp.tile([P, QC], BF16, tag="lnb", name="t")[:, 0:qn]
                        nc.vector.tensor_mul(t, x_sb[:, jj, qs], rstd)
                        nc.vector.tensor_tensor(
                            t, t, mrs, mybir.AluOpType.subtract)
                        # g*t + b on the (tail-idle) Pool engine, pipelined
                        # with the next tile's DVE ops; the store variant
                        # writes query-major so the DMA is contiguous
                        d = dest[:, qs, jj] if store else dest[:, jj, qs]
                        nc.gpsimd.tensor_scalar(
                            d, t, gv[:, jj:jj + 1],
                            bv[:, jj:jj + 1], op0=mybir.AluOpType.mult,
                            op1=mybir.AluOpType.add)
                        if dest2 is not None:
                            nc.gpsimd.tensor_scalar(
                                dest2[:, jj, qs], t, gv[:, jj:jj + 1],
                                bv2[:, jj:jj + 1], op0=mybir.AluOpType.mult,
                                op1=mybir.AluOpType.add)
                    if store:
                        nc.sync.dma_start(
                            outt_r[:, qs, :], dest[:, qs, :])

                def ffn1(c):
                    """ff1 = relu(hT @ W1.T + b1) * 8, evicted fp8 (b1
                    host-premultiplied by 8)."""
                    qs = slice(c * QC, (c + 1) * QC)
                    for m in range(NJ):
                        ps = ps_tile("ps_f1")
                        for k in range(NJ):
                            nc.tensor.matmul(
                                ps[:, 0:QC], w1_sb[:, k, m * P:(m + 1) * P],
                                hT[:, k, qs],
                                start=(k == 0), stop=(k == NJ - 1))
                        nc.scalar.activation(
                            ff1[:, m, qs], ps[:, 0:QC], AF.Relu,
                            bias=b1_sb[:, m:m + 1], scale=F1S)

                def ffn2(qs):
                    """x2 = 32*ff psum / 32 + (hT + b2) in one DVE op
                    (hTb2 pre-biased during LN1); eager square on ACT.
                    DoubleRow over ff-feature pairs."""
                    qn = qs.stop - qs.start
                    for m in range(NJ):
                        ps = ps_tile("ps_f2")
                        for kp in range(NP):
                            nc.tensor.matmul(
                                ps[:, 0:qn],
                                w2_sb[:, 2 * kp:2 * kp + 2, m * P:(m + 1) * P],
                                ff1[:, 2 * kp:2 * kp + 2, qs],
                                start=(kp == 0), stop=(kp == NP - 1),
                                perf_mode=DR,
                            )
                        nc.vector.scalar_tensor_tensor(
                            x2[:, m, qs], ps[:, 0:qn], 1.0 / (F1S * W2S),
                            hTb2[:, m, qs],
                            op0=mybir.AluOpType.mult, op1=mybir.AluOpType.add)
                        nc.scalar.activation(
                            xsq[:, m, qs], x2[:, m, qs], AF.Square)

                # software pipeline over the 2 chunks: each LN's DVE chain is
                # issued right after its stats so no engine-queue inversion,
                # and overlaps the next PE stage.  The second half of ffn2 +
                # LN2 runs in shrinking waves (128/64/64) so the serial
                # end-of-kernel chain is as short as possible.
                c0, c1 = slice(0, QC), slice(QC, NQ)
                oproj(0)
                s1m0, s1v0 = ln_stats(x1, c0, "ln1c0")
                ln_norm(x1, s1m0, s1v0, c0, g1_sb, bb1_sb, hT,
                        dest2=hTb2, bv2=bb1b2_sb)
                oproj(1)
                s1m1, s1v1 = ln_stats(x1, c1, "ln1c1")
                ln_norm(x1, s1m1, s1v1, c1, g1_sb, bb1_sb, hT,
                        dest2=hTb2, bv2=bb1b2_sb)
                ffn1(0)
                ffn2(c0)
                ffn1(1)
                s2m0, s2v0 = ln_stats(x2, c0, "ln2c0")
                ln_norm(x2, s2m0, s2v0, c0, g2_sb, bb2_sb, o_sb, store=True)
                for sq in (slice(256, 384), slice(384, 448), slice(448, 512)):
                    ffn2(sq)
                    sm, sv = ln_stats(x2, sq, f"ln2w{sq.start}")
                    ln_norm(x2, sm, sv, sq, g2_sb, bb2_sb, o_sb, store=True)

    nc.compile()
    return nc


_NC_CACHE: dict = {}


def _get_nc(nkt: int, nkp: int | None = None) -> bass.Bass:
    key = (nkt, nkp)
    if key not in _NC_CACHE:
        _NC_CACHE[key] = build_nc(nkt, nkp=nkp)
    return _NC_CACHE[key]


def _bf16(a) -> np.ndarray:
    return np.ascontiguousarray(np.asarray(a, np.float32)).astype(
        ml_dtypes.bfloat16)


def _fp8(a, scale=1.0) -> np.ndarray:
    """TRN e4m3 quantize (clip +-240) with host-side scale."""
    v = np.clip(np.asarray(a, np.float32) * np.float32(scale), -240.0, 240.0)
    return np.ascontiguousarray(v).astype(ml_dtypes.float8_e4m3fn)


def _arrange_w(wt: np.ndarray) -> np.ndarray:
    """[D, D] (in-feat, out-feat) -> [128, NJ*D] with [p, k, m] layout."""
    return np.ascontiguousarray(
        wt.reshape(NJ, P, D).transpose(1, 0, 2).reshape(P, NJ * D))


def _prep_inputs(X, Y, mask_y, Wq, Wk, Wv, Wo, ln1_g, ln1_b, ln2_g, ln2_b,
                 W1, b1, W2, b2):
    X = np.asarray(X, np.float32)
    Y = np.asarray(Y, np.float32)
    mask_y = np.asarray(mask_y)
    B = X.shape[0]

    counts = [int(mask_y[b].sum()) for b in range(B)]
    nkt = 5 if max(counts) <= 5 * P else (max(counts) + P - 1) // P
    nt = nkt * P
    nkp = min(nt, max(32, -(-max(counts) // 32) * 32))

    # transposed weights (torch Linear: x @ W.T -> lhsT rows = W.T);
    # attention weights x32 (x16 for Wo) in fp8, FFN weights bf16.
    wqt = _fp8(np.asarray(Wq, np.float32).T, WS)
    wkt = _fp8(np.asarray(Wk, np.float32).T, WS)
    wvt = _fp8(np.asarray(Wv, np.float32).T, WS)
    w1t = _bf16(np.asarray(W1, np.float32).T)
    w2t = _fp8(np.asarray(W2, np.float32).T, W2S)
    # outT tile j holds head 2j+1 in rows 0:64, head 2j in rows 64:128
    perm = np.empty(D, dtype=np.int64)
    for j in range(NJ):
        perm[j * P:j * P + HD] = (2 * j + 1) * HD + np.arange(HD)
        perm[j * P + HD:(j + 1) * P] = (2 * j) * HD + np.arange(HD)
    wot = _fp8(np.asarray(Wo, np.float32).T[perm], WS / 2.0)

    vec = lambda v: np.asarray(v, np.float32).reshape(NJ, P).T
    b1s = np.asarray(b1, np.float32) * np.float32(F1S)  # ff1 evicts at x8
    bb1b2 = np.asarray(ln1_b, np.float32) + np.asarray(b2, np.float32)
    vtail = np.concatenate(
        [vec(v) for v in (ln1_g, ln1_b, ln2_g, ln2_b, b1s, b2, bb1b2)], axis=1)
    # wv rearranged into [p, ci, k, m%256] quarters for contiguous DMAs
    wv_arr = _arrange_w(wvt).reshape(P, NJ, 4, 256).transpose(0, 2, 1, 3)
    shared = dict(
        wq=_arrange_w(wqt), wk=_arrange_w(wkt),
        wv=np.ascontiguousarray(wv_arr.reshape(P, NJ * D)),
        wo=_arrange_w(wot), w1=_arrange_w(w1t), w2=_arrange_w(w2t),
    )

    per_batch = {}
    for b in range(B):
        idx = np.flatnonzero(mask_y[b])
        nv = len(idx)
        Yc = np.zeros((nt, D), np.float32)
        bias = np.full(nt, NEG, np.float32)
        if nv == 0:
            bias[0] = 0.0   # zero sentinel key -> attn out = 0/1 = 0
        else:
            Yc[:nv] = Y[b][idx]
            bias[:nv] = 0.0
        # yt layout [p, k, key]: Yc^T[k*128+p, key]
        ytc = _fp8(Yc.T).reshape(NJ, P, nt).transpose(1, 0, 2)
        per_batch[b] = (
            np.ascontiguousarray(ytc.reshape(P, NJ * nt)),
            np.ascontiguousarray(
                np.concatenate([bias.reshape(nkt, P).T, vtail], axis=1)),
        )

    in_maps = []
    for core in range(8):
        b, half = divmod(core, 2)
        q0 = half * NQ
        m = dict(shared)
        # xt layout [p, j, q]: X^T[j*128+p, q]
        xs = X[b, q0:q0 + NQ, :].T
        m["xt"] = np.ascontiguousarray(
            _bf16(xs).reshape(NJ, P, NQ).transpose(1, 0, 2).reshape(P, NJ * NQ))
        m["xt8"] = np.ascontiguousarray(
            _fp8(xs).reshape(NJ, P, NQ).transpose(1, 0, 2).reshape(P, NJ * NQ))
        m["yt"], m["vecs"] = per_batch[b]
        in_maps.append(m)
    return in_maps, (nkt, nkp)


def unpack_output(arrs) -> np.ndarray:
    """arrs: per-core [128, NQ*NJ] f32 (query-major) -> [4, 1024, D]."""
    out = np.empty((4, 1024, D), dtype=np.float32)
    for core in range(8):
        b, half = divmod(core, 2)
        q0 = half * NQ
        a = np.asarray(arrs[core]).reshape(P, NQ, NJ)
        out[b, q0:q0 + NQ, :] = a.transpose(1, 2, 0).reshape(NQ, D)
    return out


def kernel(**inputs) -> np.ndarray:
    in_maps, (nkt, nkp) = _prep_inputs(**inputs)
    res = run_bass_kernel_spmd(_get_nc(nkt, nkp), in_maps,
                               core_ids=list(range(8)))
    return unpack_output([res.results[c]["outt"] for c in range(8)])


# revision 27
# speedup vs baseline: 1.0854x; 1.0854x over previous
"""Trainium2 Bass kernel v3 for the dense transformer block (cross-attn + FFN).

Problem: nn_MAB (B=4, nq=nk=1024, D=1024, H=16, HD=64), fp32 in/out.

Sharding: data-parallel, zero collectives. 8 cores = 4 batches x 2
query-halves; each core computes 512 query rows of one batch end-to-end.

v3 changes vs v2 (bf16 baseline, ~146us):
  - fp8(e4m3) attention path: Y/X/Wq/Wk/Wv/Wo quantized to TRN e4m3
    (clip +-240).  Weights host-scaled x32 (x16 for Wo) to stay clear of
    fp8 subnormals; the compensating 1/8192 folds into the exp activation
    scale and 1/512 into the O-proj eviction.  Numpy end-to-end emulation:
    rel err 4.1e-3 vs 3.9e-3 all-bf16 (tol 2e-2) -- attention output is
    ~3% of the residual stream, so its quantization noise is negligible.
  - DoubleRow perf mode (2 fp8 MACs/cell/cycle) for the contraction-1024
    matmuls: V/K/Q projections and the O projection.  Scores (K=64) and
    attnV (M=65 ones-column denominator trick) stay normal-mode fp8
    (1 cyc/row, same as bf16).
  - X residual path stays bf16 (fp8 there would put 3.6% straight into
    the output); FFN stays bf16.
  - yt layout changed to [P, NJ(feat-block), nt(keys)] so the same SBUF
    tile serves V-proj DR lhsT ([128,2,128] feat-pair slices) and K-proj
    DR rhs ([128,2,<=256] key chunks).

Measured v2: 145779ns harness; v3 target ~120us (PE busy 133->~110us).
"""

import numpy as np
import ml_dtypes

import concourse.bass as bass
import concourse.mybir as mybir
import concourse.tile as tile
from concourse import bacc
from concourse.bass_utils import run_bass_kernel_spmd

F32 = mybir.dt.float32
F32R = mybir.dt.float32r
BF16 = mybir.dt.bfloat16
F8 = mybir.dt.float8e4
AF = mybir.ActivationFunctionType
DR = mybir.MatmulPerfMode.DoubleRow

D = 1024          # model dim
P = 128           # partitions
NJ = D // P       # feature tiles (8)
NP = NJ // 2      # feature-pair tiles for DoubleRow (4)
NQ = 512          # queries per core
H = 16
HD = 64
NPAIR = H // 2    # head pairs (8)
NEG = -30000.0    # additive mask for dropped/pad keys
EPS = 1e-5
QC = 256          # tail query-chunk
NQC = NQ // QC    # tail chunks (2)
WS = 32.0         # fp8 weight scale (wq/wk/wv); wo uses WS/2
ESC = 1.0 / (WS * WS * 8.0)   # exp activation scale (1/8192)
OSC = 1.0 / (WS * WS / 2.0)   # o-proj eviction scale (1/512)
F1S = 8.0         # ff1 relu eviction scale (fp8; b1 host-premultiplied)
W2S = 4.0         # W2 fp8 host scale -> ffn2 psum = 32*ff


def build_nc(nkt: int, reps: int = 1, nkp: int | None = None) -> bass.Bass:
    """nkt = number of 128-key tiles (5 for the compacted fast path).
    nkp = key count the K-projection actually computes (multiple of 32,
    >= max valid keys; the kt pad region [nkp:nt] is zeroed so masked
    scores stay finite). reps > 1 wraps the body in a hardware loop
    (timing amplification only)."""
    from contextlib import nullcontext
    nt = nkt * P
    if nkp is None:
        nkp = nt
    assert nkp % 32 == 0 and 0 < nkp <= nt
    nc = bacc.Bacc("TRN2", target_bir_lowering=False, debug=False)

    # ---- DRAM I/O (per-core shards; host prepares exact SBUF layouts) ----
    xt = nc.dram_tensor("xt", [P, NJ * NQ], BF16, kind="ExternalInput")[:]
    xt8 = nc.dram_tensor("xt8", [P, NJ * NQ], F8, kind="ExternalInput")[:]
    yt = nc.dram_tensor("yt", [P, NJ * nt], F8, kind="ExternalInput")[:]
    wq = nc.dram_tensor("wq", [P, NJ * D], F8, kind="ExternalInput")[:]
    wk = nc.dram_tensor("wk", [P, NJ * D], F8, kind="ExternalInput")[:]
    wv = nc.dram_tensor("wv", [P, NJ * D], F8, kind="ExternalInput")[:]
    wo = nc.dram_tensor("wo", [P, NJ * D], F8, kind="ExternalInput")[:]
    w1 = nc.dram_tensor("w1", [P, NJ * D], BF16, kind="ExternalInput")[:]
    w2 = nc.dram_tensor("w2", [P, NJ * D], F8, kind="ExternalInput")[:]
    # one [p, nkt + 7*NJ] tensor: mask bias, then g1/bb1/g2/bb2/b1/b2/bb1b2
    vecs = nc.dram_tensor("vecs", [P, nkt + 7 * NJ], F32, kind="ExternalInput")[:]
    outt = nc.dram_tensor("outt", [P, NJ * NQ], F32, kind="ExternalOutput")[:]

    with tile.TileContext(nc) as tc, \
         nc.allow_low_precision(reason="fp8 attention path; tol 2e-2"), \
         (tc.For_i(0, reps) if reps > 1 else nullcontext()), \
         tc.tile_pool(name="persist", bufs=1) as persist, \
         tc.tile_pool(name="psum", bufs=8, space="PSUM") as pp:
        if True:

            def ps_tile(name):
                return pp.tile([P, NQ], F32, tag="ps", name=name)

            yt_sb = persist.tile([P, NJ, nt], F8)
            wv_sb = persist.tile([P, 4, NJ, 256], F8)  # [ci, k, m%256] quarters
            wk_sb = persist.tile([P, NJ, D], F8)
            wq_sb = persist.tile([P, NJ, D], F8)
            wo_sb = persist.tile([P, NJ, D], F8)
            w1_sb = persist.tile([P, NJ, D], BF16)
            w2_sb = persist.tile([P, NJ, D], F8)
            xt_sb = persist.tile([P, NJ, NQ], BF16)
            xt8_sb = persist.tile([P, NJ, NQ], F8)

            # ---- small vectors first (mask_sb gates the first exp) ----
            vec_sb = persist.tile([P, nkt + 7 * NJ], F32)
            nc.sync.dma_start(vec_sb, vecs)
            mask_sb = vec_sb[:, 0:nkt]
            g1_sb = vec_sb[:, nkt + 0 * NJ:nkt + 1 * NJ]
            bb1_sb = vec_sb[:, nkt + 1 * NJ:nkt + 2 * NJ]
            g2_sb = vec_sb[:, nkt + 2 * NJ:nkt + 3 * NJ]
            bb2_sb = vec_sb[:, nkt + 3 * NJ:nkt + 4 * NJ]
            b1_sb = vec_sb[:, nkt + 4 * NJ:nkt + 5 * NJ]
            b2_sb = vec_sb[:, nkt + 5 * NJ:nkt + 6 * NJ]
            bb1b2_sb = vec_sb[:, nkt + 6 * NJ:nkt + 7 * NJ]  # ln1_b + b2

            # big loads, all on the SP queue in first-use order, each one
            # fully contiguous (strided DMAs pay ~2us of per-row descriptor
            # overhead).  wv is host-arranged in [ci, k, m%256] quarters so
            # the first V matmul waits on yt + a quarter of Wv only.
            nc.sync.dma_start(yt_sb, yt.rearrange("p (k m) -> p k m", k=NJ))
            wv_r = wv.rearrange("p (c k m) -> p c k m", c=4, k=NJ)
            for ci in range(4):
                nc.sync.dma_start(wv_sb[:, ci], wv_r[:, ci])
            nc.sync.dma_start(wk_sb, wk.rearrange("p (k m) -> p k m", k=NJ))
            nc.sync.dma_start(xt8_sb, xt8.rearrange("p (j q) -> p j q", j=NJ))
            nc.sync.dma_start(wq_sb, wq.rearrange("p (k m) -> p k m", k=NJ))
            nc.sync.dma_start(wo_sb, wo.rearrange("p (k m) -> p k m", k=NJ))
            nc.sync.dma_start(xt_sb, xt.rearrange("p (j q) -> p j q", j=NJ))
            nc.sync.dma_start(w1_sb, w1.rearrange("p (k m) -> p k m", k=NJ))
            nc.sync.dma_start(w2_sb, w2.rearrange("p (k m) -> p k m", k=NJ))

            # ---- constants ----
            lnw = persist.tile([P, P], BF16)         # 1/D for LN stat matmuls
            nc.vector.memset(lnw, 1.0 / D)
            eps_sb = persist.tile([P, 1], F32)
            nc.vector.memset(eps_sb, EPS)

            # attention head outputs (x32 scale), feature-major; tile j rows
            # 0:64 = head 2j+1, rows 64:128 = head 2j (host permutes Wo rows).
            outT = persist.tile([P, NJ, NQ], F8)

            with tc.tile_pool(name="attn", bufs=1) as big:
                # V (x32) with an interleaved ones column per head
                v_sb = big.tile([P, nkt, H, HD + 1], F8)
                nc.vector.memset(v_sb[:, :, :, HD:HD + 1], 1.0)

                # ---- V = Y @ Wv.T x32 (DoubleRow over feature pairs),
                # quarter-chunks so the first matmul only waits for the
                # yt pairs + a quarter of Wv ----
                for ci in range(4):
                    for t in range(nkt):
                        ps = ps_tile("ps_v")
                        for kp in range(NP):
                            nc.tensor.matmul(
                                ps[:, 0:256],
                                yt_sb[:, 2 * kp:2 * kp + 2, t * P:(t + 1) * P],
                                wv_sb[:, ci, 2 * kp:2 * kp + 2, :],
                                start=(kp == 0), stop=(kp == NP - 1),
                                perf_mode=DR,
                            )
                        nc.vector.tensor_copy(
                            v_sb[:, t, ci * 4:(ci + 1) * 4, 0:HD], ps[:, 0:256])

                # ---- per head-pair: KT, QT, scoresT, exp, attnV ----
                with tc.tile_pool(name="qk", bufs=2) as qkp, \
                     tc.tile_pool(name="exp", bufs=4) as ep, \
                     tc.tile_pool(name="stage", bufs=3) as stp:
                    deferred = []   # (ps_e, ps_o, rc_e, rc_o, j) of prev pair

                    def flush_deferred():
                        for (pse, pso, rce, rco, jj) in deferred:
                            # replicate the reciprocal rows across 64
                            # partitions on the (idle) Pool engine.  HW ucode
                            # broadcasts the physical partition 0, so first
                            # hop the row from partition 64 to 0 via a tiny
                            # shift DMA.
                            rc0o = stp.tile([P, NQ], F32R, tag="rc0", name="rc0o")
                            nc.sync.dma_start(rc0o[0:1, :], rco[HD:HD + 1, :])
                            rc0e = stp.tile([P, NQ], F32R, tag="rc0", name="rc0e")
                            nc.sync.dma_start(rc0e[0:1, :], rce[HD:HD + 1, :])
                            rcb_o = stp.tile([P, NQ], F32R, tag="rcb", name="rcb_o")
                            nc.gpsimd.partition_broadcast(
                                rcb_o[0:HD, :], rc0o[0:1, :])
                            rcb_e = stp.tile([P, NQ], F32R, tag="rcb", name="rcb_e")
                            nc.gpsimd.partition_broadcast(
                                rcb_e[0:HD, :], rc0e[0:1, :])
                            # odd head -> outT rows 0:64 directly (x32 kept)
                            nc.vector.tensor_mul(
                                outT[0:HD, jj, :], pso[0:HD, :], rcb_o[0:HD, :])
                            # even head -> staging, partition-shift to 64:128
                            tmp = stp.tile([P, NQ], F8, tag="tmp", name="tmp")
                            nc.vector.tensor_mul(
                                tmp[0:HD, :], pse[0:HD, :], rcb_e[0:HD, :])
                            nc.sync.dma_start(outT[HD:P, jj, :], tmp[0:HD, :])
                        deferred.clear()

                    def emit_kq(j):
                        """K^T + Q^T matmuls for pair j -> (kt_j, qt_j), both
                        x32-scaled fp8.  Streams only nkp keys; pad region
                        zeroed on the first ring pass."""
                        ms = slice(j * P, (j + 1) * P)
                        kt_j = qkp.tile([P, nt], F8, tag="kt", name="kt_j")
                        if nkp < nt:
                            nc.vector.memset(kt_j[:, nkp:nt], 0.0)
                        # DoubleRow chunks of 256 keys, then a <=224 tail
                        # (normal mode: DR loses below 128 free).
                        chunks = []
                        k0 = 0
                        while k0 < nkp:
                            kn = min(256, nkp - k0)
                            chunks.append((k0, kn))
                            k0 += kn
                        for (k0, kn) in chunks:
                            ks = slice(k0, k0 + kn)
                            ps = ps_tile("ps_k")
                            if kn >= P:
                                for kp in range(NP):
                                    nc.tensor.matmul(
                                        ps[:, 0:kn],
                                        wk_sb[:, 2 * kp:2 * kp + 2, ms],
                                        yt_sb[:, 2 * kp:2 * kp + 2, ks],
                                        start=(kp == 0), stop=(kp == NP - 1),
                                        perf_mode=DR,
                                    )
                            else:
                                for k in range(NJ):
                                    nc.tensor.matmul(
                                        ps[:, 0:kn], wk_sb[:, k, ms],
                                        yt_sb[:, k, ks],
                                        start=(k == 0), stop=(k == NJ - 1),
                                    )
                            nc.vector.tensor_copy(kt_j[:, ks], ps[:, 0:kn])
                        # Q^T m-tile j (x32; /8 folded into exp scale)
                        qt_j = qkp.tile([P, NQ], F8, tag="qt", name="qt_j")
                        for c in range(2):
                            qs = slice(c * 256, (c + 1) * 256)
                            ps = ps_tile("ps_q")
                            for kp in range(NP):
                                nc.tensor.matmul(
                                    ps[:, 0:256],
                                    wq_sb[:, 2 * kp:2 * kp + 2, ms],
                                    xt8_sb[:, 2 * kp:2 * kp + 2, qs],
                                    start=(kp == 0), stop=(kp == NP - 1),
                                    perf_mode=DR,
                                )
                            nc.vector.tensor_copy(qt_j[:, qs], ps[:, 0:256])
                        return kt_j, qt_j

                    def emit_scores(j, kt_j, qt_j):
                        """scoresT + exp for pair j, row-packed on the PE.
                        psum = 8192*s; exp applies scale=1/8192 + mask bias."""
                        exp_e = ep.tile([P, nkt, NQ], F8, tag="exp", name="exp_e")
                        exp_o = ep.tile([P, nkt, NQ], F8, tag="exp", name="exp_o")
                        for t in range(nkt):
                            ks = slice(t * P, (t + 1) * P)
                            ps0 = ps_tile("ps_s0")
                            nc.tensor.matmul(
                                ps0, kt_j[0:HD, ks], qt_j[0:HD, :],
                                start=True, stop=True, tile_position=(0, 0),
                            )
                            ps1 = ps_tile("ps_s1")
                            nc.tensor.matmul(
                                ps1, kt_j[HD:P, ks], qt_j[HD:P, :],
                                start=True, stop=True, tile_position=(HD, 0),
                            )
                            nc.scalar.activation(
                                exp_e[:, t, :], ps0, AF.Exp,
                                bias=mask_sb[:, t:t + 1], scale=ESC)
                            nc.scalar.activation(
                                exp_o[:, t, :], ps1, AF.Exp,
                                bias=mask_sb[:, t:t + 1], scale=ESC)
                        return exp_e, exp_o

                    def emit_attnv(j, exp_e, exp_o):
                        """attnV: lhsT = [V_h*32 | ones] (M=65) -> rows 0:64
                        x32 numerator, row 64 = softmax denominator."""
                        ps_e = ps_tile("ps_ae")
                        ps_o = ps_tile("ps_ao")
                        for t in range(nkt):
                            st, sp = t == 0, t == nkt - 1
                            nc.tensor.matmul(
                                ps_e[0:HD + 1, :], v_sb[:, t, 2 * j, :],
                                exp_e[:, t, :], start=st, stop=sp,
                            )
                        for t in range(nkt):
                            st, sp = t == 0, t == nkt - 1
                            nc.tensor.matmul(
                                ps_o[0:HD + 1, :], v_sb[:, t, 2 * j + 1, :],
                                exp_o[:, t, :], start=st, stop=sp,
                            )
                        # reciprocals (partition 64, f32r) on DVE now; divide
                        # + broadcast run one pair later via flush_deferred
                        rc_e = stp.tile([P, NQ], F32R, tag="rc", name="rc_e")
                        nc.vector.reciprocal(
                            rc_e[HD:HD + 1, :], ps_e[HD:HD + 1, :])
                        rc_o = stp.tile([P, NQ], F32R, tag="rc", name="rc_o")
                        nc.vector.reciprocal(
                            rc_o[HD:HD + 1, :], ps_o[HD:HD + 1, :])
                        deferred.append((ps_e, ps_o, rc_e, rc_o, j))

                    # software pipeline: next pair's K/Q matmuls are issued
                    # between this pair's scores and attnV so PE never waits
                    # on the DVE evictions or the exp activations.
                    kq = emit_kq(0)
                    for j in range(NPAIR):
                        flush_deferred()
                        exps = emit_scores(j, *kq)
                        if j + 1 < NPAIR:
                            kq = emit_kq(j + 1)
                        emit_attnv(j, *exps)
                    flush_deferred()

            # ---- tail: O-proj + LN1 + FFN + LN2, 2 query-chunks ----
            with tc.tile_pool(name="tail", bufs=1) as tl, \
                 tc.tile_pool(name="ln", bufs=8) as lnp:

                x1 = tl.tile([P, NJ, NQ], BF16)
                xsq = tl.tile([P, NJ, NQ], BF16)
                hT = tl.tile([P, NJ, NQ], BF16)
                hTb2 = tl.tile([P, NJ, NQ], BF16)
                ff1 = tl.tile([P, NJ, NQ], F8)
                x2 = tl.tile([P, NJ, NQ], BF16)
                o_sb = tl.tile([P, NQ, NJ], F32)   # query-major for the store

                def oproj(c):
                    """Z = outT.T @ Wo.T via DoubleRow over feature pairs;
                    psum = 512*Z, evicted with a 1/512 scale + X residual."""
                    qs = slice(c * QC, (c + 1) * QC)
                    for m in range(NJ):
                        ps = ps_tile("ps_z")
                        for gp in range(NP):
                            nc.tensor.matmul(
                                ps[:, 0:QC],
                                wo_sb[:, 2 * gp:2 * gp + 2, m * P:(m + 1) * P],
                                outT[:, 2 * gp:2 * gp + 2, qs],
                                start=(gp == 0), stop=(gp == NP - 1),
                                perf_mode=DR,
                            )
                        nc.vector.scalar_tensor_tensor(
                            x1[:, m, qs], ps[:, 0:QC], OSC, xt_sb[:, m, qs],
                            op0=mybir.AluOpType.mult, op1=mybir.AluOpType.add)
                        # eager square so LN1 stats don't wait on ACT later
                        nc.scalar.activation(
                            xsq[:, m, qs], x1[:, m, qs], AF.Square)

                def ln_stats(x_sb, qs, name):
                    """mean/E[x^2] matmuls for query-slice qs (squares
                    already in xsq, computed eagerly at eviction)."""
                    qn = qs.stop - qs.start
                    ps_m = ps_tile(name + "_m")
                    for jj in range(NJ):
                        nc.tensor.matmul(
                            ps_m[:, 0:qn], lnw, x_sb[:, jj, qs],
                            start=(jj == 0), stop=(jj == NJ - 1))
                    ps_v = ps_tile(name + "_v")
                    for jj in range(NJ):
                        nc.tensor.matmul(
                            ps_v[:, 0:qn], lnw, xsq[:, jj, qs],
                            start=(jj == 0), stop=(jj == NJ - 1))
                    return ps_m, ps_v

                # output is query-major [p, q, j] so each wave's store is one
                # fully-contiguous DMA (strided stores pay the ~2.3us
                # 128-descriptor floor per call)
                outt_r = outt.rearrange("p (q j) -> p q j", j=NJ)

                def ln_norm(x_sb, ps_m, ps_v, qs, gv, bv, dest, store=False,
                            dest2=None, bv2=None):
                    """DVE/Pool chain: normalize query-slice qs into dest.
                    Stats psums are read in place (no mean eviction); rstd
                    comes from one ACT Rsqrt.  dest2/bv2 emit a second
                    biased copy (hT + b2 for the ffn2 eviction) on Pool."""
                    qn = qs.stop - qs.start
                    mean = lnp.tile([P, QC], F32, tag="lnt", name="mean")[:, 0:qn]
                    nc.vector.tensor_copy(mean, ps_m[:, 0:qn])
                    var = lnp.tile([P, QC], F32, tag="lnt", name="var")[:, 0:qn]
                    nc.vector.tensor_mul(var, mean, mean)
                    nc.vector.tensor_tensor(
                        var, ps_v[:, 0:qn], var, mybir.AluOpType.subtract)
                    sd = lnp.tile([P, QC], F32, tag="lnt", name="sd")[:, 0:qn]
                    nc.scalar.activation(sd, var, AF.Sqrt, bias=eps_sb, scale=1.0)
                    rstd = lnp.tile([P, QC], BF16, tag="lnr", name="rstd")[:, 0:qn]
                    nc.vector.reciprocal(rstd, sd)
                    mrs = lnp.tile([P, QC], BF16, tag="lnr", name="mrs")[:, 0:qn]
                    nc.vector.tensor_mul(mrs, mean, rstd)
                    for jj in range(NJ):
                        t = lnp.tile([P, QC], BF16, tag="lnb", name="t")[:, 0:qn]
                        nc.vector.tensor_mul(t, x_sb[:, jj, qs], rstd)
                        nc.vector.tensor_tensor(
                            t, t, mrs, mybir.AluOpType.subtract)
                        # g*t + b on the (tail-idle) ACT engine, pipelined
                        # with the next tile's DVE ops; the store variant
                        # writes query-major so the DMA is contiguous
                        d = dest[:, qs, jj] if store else dest[:, jj, qs]
                        nc.scalar.activation(
                            d, t, AF.Identity,
                            bias=bv[:, jj:jj + 1], scale=gv[:, jj:jj + 1])
                        if dest2 is not None:
                            nc.scalar.activation(
                                dest2[:, jj, qs], t, AF.Identity,
                                bias=bv2[:, jj:jj + 1], scale=gv[:, jj:jj + 1])
                    if store:
                        nc.sync.dma_start(
                            outt_r[:, qs, :], dest[:, qs, :])

                def ffn1(c):
                    """ff1 = relu(hT @ W1.T + b1) * 8, evicted fp8 (b1
                    host-premultiplied by 8)."""
                    qs = slice(c * QC, (c + 1) * QC)
                    for m in range(NJ):
                        ps = ps_tile("ps_f1")
                        for k in range(NJ):
                            nc.tensor.matmul(
                                ps[:, 0:QC], w1_sb[:, k, m * P:(m + 1) * P],
                                hT[:, k, qs],
                                start=(k == 0), stop=(k == NJ - 1))
                        nc.scalar.activation(
                            ff1[:, m, qs], ps[:, 0:QC], AF.Relu,
                            bias=b1_sb[:, m:m + 1], scale=F1S)

                def ffn2(qs):
                    """x2 = 32*ff psum / 32 + (hT + b2) in one DVE op
                    (hTb2 pre-biased during LN1); eager square on ACT.
                    DoubleRow over ff-feature pairs."""
                    qn = qs.stop - qs.start
                    for m in range(NJ):
                        ps = ps_tile("ps_f2")
                        for kp in range(NP):
                            nc.tensor.matmul(
                                ps[:, 0:qn],
                                w2_sb[:, 2 * kp:2 * kp + 2, m * P:(m + 1) * P],
                                ff1[:, 2 * kp:2 * kp + 2, qs],
                                start=(kp == 0), stop=(kp == NP - 1),
                                perf_mode=DR,
                            )
                        nc.vector.scalar_tensor_tensor(
                            x2[:, m, qs], ps[:, 0:qn], 1.0 / (F1S * W2S),
                            hTb2[:, m, qs],
                            op0=mybir.AluOpType.mult, op1=mybir.AluOpType.add)
                        nc.scalar.activation(
                            xsq[:, m, qs], x2[:, m, qs], AF.Square)

                # software pipeline over the 2 chunks: each LN's DVE chain is
                # issued right after its stats so no engine-queue inversion,
                # and overlaps the next PE stage.  The second half of ffn2 +
                # LN2 runs in shrinking waves (128/64/64) so the serial
                # end-of-kernel chain is as short as possible.
                c0, c1 = slice(0, QC), slice(QC, NQ)
                oproj(0)
                s1m0, s1v0 = ln_stats(x1, c0, "ln1c0")
                ln_norm(x1, s1m0, s1v0, c0, g1_sb, bb1_sb, hT,
                        dest2=hTb2, bv2=bb1b2_sb)
                oproj(1)
                s1m1, s1v1 = ln_stats(x1, c1, "ln1c1")
                ln_norm(x1, s1m1, s1v1, c1, g1_sb, bb1_sb, hT,
                        dest2=hTb2, bv2=bb1b2_sb)
                ffn1(0)
                ffn2(c0)
                ffn1(1)
                s2m0, s2v0 = ln_stats(x2, c0, "ln2c0")
                ln_norm(x2, s2m0, s2v0, c0, g2_sb, bb2_sb, o_sb, store=True)
                for sq in (slice(256, 384), slice(384, 448), slice(448, 512)):
                    ffn2(sq)
                    sm, sv = ln_stats(x2, sq, f"ln2w{sq.start}")
                    ln_norm(x2, sm, sv, sq, g2_sb, bb2_sb, o_sb, store=True)

    nc.compile()
    return nc


_NC_CACHE: dict = {}


def _get_nc(nkt: int, nkp: int | None = None) -> bass.Bass:
    key = (nkt, nkp)
    if key not in _NC_CACHE:
        _NC_CACHE[key] = build_nc(nkt, nkp=nkp)
    return _NC_CACHE[key]


def _bf16(a) -> np.ndarray:
    return np.ascontiguousarray(np.asarray(a, np.float32)).astype(
        ml_dtypes.bfloat16)


def _fp8(a, scale=1.0) -> np.ndarray:
    """TRN e4m3 quantize (clip +-240) with host-side scale."""
    v = np.clip(np.asarray(a, np.float32) * np.float32(scale), -240.0, 240.0)
    return np.ascontiguousarray(v).astype(ml_dtypes.float8_e4m3fn)


def _arrange_w(wt: np.ndarray) -> np.ndarray:
    """[D, D] (in-feat, out-feat) -> [128, NJ*D] with [p, k, m] layout."""
    return np.ascontiguousarray(
        wt.reshape(NJ, P, D).transpose(1, 0, 2).reshape(P, NJ * D))


def _prep_inputs(X, Y, mask_y, Wq, Wk, Wv, Wo, ln1_g, ln1_b, ln2_g, ln2_b,
                 W1, b1, W2, b2):
    X = np.asarray(X, np.float32)
    Y = np.asarray(Y, np.float32)
    mask_y = np.asarray(mask_y)
    B = X.shape[0]

    counts = [int(mask_y[b].sum()) for b in range(B)]
    nkt = 5 if max(counts) <= 5 * P else (max(counts) + P - 1) // P
    nt = nkt * P
    nkp = min(nt, max(32, -(-max(counts) // 32) * 32))

    # transposed weights (torch Linear: x @ W.T -> lhsT rows = W.T);
    # attention weights x32 (x16 for Wo) in fp8, FFN weights bf16.
    wqt = _fp8(np.asarray(Wq, np.float32).T, WS)
    wkt = _fp8(np.asarray(Wk, np.float32).T, WS)
    wvt = _fp8(np.asarray(Wv, np.float32).T, WS)
    w1t = _bf16(np.asarray(W1, np.float32).T)
    w2t = _fp8(np.asarray(W2, np.float32).T, W2S)
    # outT tile j holds head 2j+1 in rows 0:64, head 2j in rows 64:128
    perm = np.empty(D, dtype=np.int64)
    for j in range(NJ):
        perm[j * P:j * P + HD] = (2 * j + 1) * HD + np.arange(HD)
        perm[j * P + HD:(j + 1) * P] = (2 * j) * HD + np.arange(HD)
    wot = _fp8(np.asarray(Wo, np.float32).T[perm], WS / 2.0)

    vec = lambda v: np.asarray(v, np.float32).reshape(NJ, P).T
    b1s = np.asarray(b1, np.float32) * np.float32(F1S)  # ff1 evicts at x8
    bb1b2 = np.asarray(ln1_b, np.float32) + np.asarray(b2, np.float32)
    vtail = np.concatenate(
        [vec(v) for v in (ln1_g, ln1_b, ln2_g, ln2_b, b1s, b2, bb1b2)], axis=1)
    # wv rearranged into [p, ci, k, m%256] quarters for contiguous DMAs
    wv_arr = _arrange_w(wvt).reshape(P, NJ, 4, 256).transpose(0, 2, 1, 3)
    shared = dict(
        wq=_arrange_w(wqt), wk=_arrange_w(wkt),
        wv=np.ascontiguousarray(wv_arr.reshape(P, NJ * D)),
        wo=_arrange_w(wot), w1=_arrange_w(w1t), w2=_arrange_w(w2t),
    )

    per_batch = {}
    for b in range(B):
        idx = np.flatnonzero(mask_y[b])
        nv = len(idx)
        Yc = np.zeros((nt, D), np.float32)
        bias = np.full(nt, NEG, np.float32)
        if nv == 0:
            bias[0] = 0.0   # zero sentinel key -> attn out = 0/1 = 0
        else:
            Yc[:nv] = Y[b][idx]
            bias[:nv] = 0.0
        # yt layout [p, k, key]: Yc^T[k*128+p, key]
        ytc = _fp8(Yc.T).reshape(NJ, P, nt).transpose(1, 0, 2)
        per_batch[b] = (
            np.ascontiguousarray(ytc.reshape(P, NJ * nt)),
            np.ascontiguousarray(
                np.concatenate([bias.reshape(nkt, P).T, vtail], axis=1)),
        )

    in_maps = []
    for core in range(8):
        b, half = divmod(core, 2)
        q0 = half * NQ
        m = dict(shared)
        # xt layout [p, j, q]: X^T[j*128+p, q]
        xs = X[b, q0:q0 + NQ, :].T
        m["xt"] = np.ascontiguousarray(
            _bf16(xs).reshape(NJ, P, NQ).transpose(1, 0, 2).reshape(P, NJ * NQ))
        m["xt8"] = np.ascontiguousarray(
            _fp8(xs).reshape(NJ, P, NQ).transpose(1, 0, 2).reshape(P, NJ * NQ))
        m["yt"], m["vecs"] = per_batch[b]
        in_maps.append(m)
    return in_maps, (nkt, nkp)


def unpack_output(arrs) -> np.ndarray:
    """arrs: per-core [128, NQ*NJ] f32 (query-major) -> [4, 1024, D]."""
    out = np.empty((4, 1024, D), dtype=np.float32)
    for core in range(8):
        b, half = divmod(core, 2)
        q0 = half * NQ
        a = np.asarray(arrs[core]).reshape(P, NQ, NJ)
        out[b, q0:q0 + NQ, :] = a.transpose(1, 2, 0).reshape(NQ, D)
    return out


def kernel(**inputs) -> np.ndarray:
    in_maps, (nkt, nkp) = _prep_inputs(**inputs)
    res = run_bass_kernel_spmd(_get_nc(nkt, nkp), in_maps,
                               core_ids=list(range(8)))
    return unpack_output([res.results[c]["outt"] for c in range(8)])


# revision 36
# speedup vs baseline: 1.1059x; 1.0189x over previous
"""Trainium2 Bass kernel v3 for the dense transformer block (cross-attn + FFN).

Problem: nn_MAB (B=4, nq=nk=1024, D=1024, H=16, HD=64), fp32 in/out.

Sharding: data-parallel, zero collectives. 8 cores = 4 batches x 2
query-halves; each core computes 512 query rows of one batch end-to-end.

v3 changes vs v2 (bf16 baseline, ~146us):
  - fp8(e4m3) attention path: Y/X/Wq/Wk/Wv/Wo quantized to TRN e4m3
    (clip +-240).  Weights host-scaled x32 (x16 for Wo) to stay clear of
    fp8 subnormals; the compensating 1/8192 folds into the exp activation
    scale and 1/512 into the O-proj eviction.  Numpy end-to-end emulation:
    rel err 4.1e-3 vs 3.9e-3 all-bf16 (tol 2e-2) -- attention output is
    ~3% of the residual stream, so its quantization noise is negligible.
  - DoubleRow perf mode (2 fp8 MACs/cell/cycle) for the contraction-1024
    matmuls: V/K/Q projections and the O projection.  Scores (K=64) and
    attnV (M=65 ones-column denominator trick) stay normal-mode fp8
    (1 cyc/row, same as bf16).
  - X residual path stays bf16 (fp8 there would put 3.6% straight into
    the output); FFN stays bf16.
  - yt layout changed to [P, NJ(feat-block), nt(keys)] so the same SBUF
    tile serves V-proj DR lhsT ([128,2,128] feat-pair slices) and K-proj
    DR rhs ([128,2,<=256] key chunks).

Measured v2: 145779ns harness; v3 target ~120us (PE busy 133->~110us).
"""

import numpy as np
import ml_dtypes

import concourse.bass as bass
import concourse.mybir as mybir
import concourse.tile as tile
from concourse import bacc
from concourse.bass_utils import run_bass_kernel_spmd

F32 = mybir.dt.float32
F32R = mybir.dt.float32r
BF16 = mybir.dt.bfloat16
F8 = mybir.dt.float8e4
AF = mybir.ActivationFunctionType
DR = mybir.MatmulPerfMode.DoubleRow

D = 1024          # model dim
P = 128           # partitions
NJ = D // P       # feature tiles (8)
NP = NJ // 2      # feature-pair tiles for DoubleRow (4)
NQ = 512          # queries per core
H = 16
HD = 64
NPAIR = H // 2    # head pairs (8)
NEG = -30000.0    # additive mask for dropped/pad keys
EPS = 1e-5
QC = 256          # tail query-chunk
NQC = NQ // QC    # tail chunks (2)
WS = 32.0         # fp8 weight scale (wq/wk/wv); wo uses WS/2
ESC = 1.0 / (WS * WS * 8.0)   # exp activation scale (1/8192)
OSC = 1.0 / (WS * WS / 2.0)   # o-proj eviction scale (1/512)
F1S = 8.0         # ff1 relu eviction scale (fp8; b1 host-premultiplied)
W2S = 4.0         # W2 fp8 host scale -> ffn2 psum = 32*ff


def build_nc(nkt: int, reps: int = 1, nkp: int | None = None) -> bass.Bass:
    """nkt = number of 128-key tiles (5 for the compacted fast path).
    nkp = key count the K-projection actually computes (multiple of 32,
    >= max valid keys; the kt pad region [nkp:nt] is zeroed so masked
    scores stay finite). reps > 1 wraps the body in a hardware loop
    (timing amplification only)."""
    from contextlib import nullcontext
    nt = nkt * P
    if nkp is None:
        nkp = nt
    assert nkp % 32 == 0 and 0 < nkp <= nt
    nc = bacc.Bacc("TRN2", target_bir_lowering=False, debug=False)

    # ---- DRAM I/O (per-core shards; host prepares exact SBUF layouts) ----
    xt = nc.dram_tensor("xt", [P, NJ * NQ], BF16, kind="ExternalInput")[:]
    xt8 = nc.dram_tensor("xt8", [P, NJ * NQ], F8, kind="ExternalInput")[:]
    yt = nc.dram_tensor("yt", [P, NJ * nt], F8, kind="ExternalInput")[:]
    wq = nc.dram_tensor("wq", [P, NJ * D], F8, kind="ExternalInput")[:]
    wk = nc.dram_tensor("wk", [P, NJ * D], F8, kind="ExternalInput")[:]
    wv = nc.dram_tensor("wv", [P, NJ * D], F8, kind="ExternalInput")[:]
    wo = nc.dram_tensor("wo", [P, NJ * D], F8, kind="ExternalInput")[:]
    w1 = nc.dram_tensor("w1", [P, NJ * D], BF16, kind="ExternalInput")[:]
    w2 = nc.dram_tensor("w2", [P, NJ * D], F8, kind="ExternalInput")[:]
    # one [p, nkt + 7*NJ] tensor: mask bias, then g1/bb1/g2/bb2/b1/b2/bb1b2
    vecs = nc.dram_tensor("vecs", [P, nkt + 7 * NJ], F32, kind="ExternalInput")[:]
    outt = nc.dram_tensor("outt", [P, NJ * NQ], F32, kind="ExternalOutput")[:]

    import os
    hoist = reps > 1 and os.environ.get("HOIST", "0") == "1"

    with tile.TileContext(nc) as tc, \
         nc.allow_low_precision(reason="fp8 attention path; tol 2e-2"), \
         tc.tile_pool(name="persist", bufs=1) as persist, \
         tc.tile_pool(name="psum", bufs=8, space="PSUM") as pp:

            def ps_tile(name):
                return pp.tile([P, NQ], F32, tag="ps", name=name)

            yt_sb = persist.tile([P, NJ, nt], F8)
            wv_sb = persist.tile([P, 2, NJ, 512], F8)  # [ci, k, m%512] halves
            wk_sb = persist.tile([P, NJ, D], F8)
            wq_sb = persist.tile([P, NJ, D], F8)
            wo_sb = persist.tile([P, NJ, D], F8)
            w1_sb = persist.tile([P, NJ, D], BF16)
            w2_sb = persist.tile([P, NJ, D], F8)
            xt_sb = persist.tile([P, NJ, NQ], BF16)
            xt8_sb = persist.tile([P, NJ, NQ], F8)
            vec_sb = persist.tile([P, nkt + 7 * NJ], F32)
            mask_sb = vec_sb[:, 0:nkt]
            g1_sb = vec_sb[:, nkt + 0 * NJ:nkt + 1 * NJ]
            bb1_sb = vec_sb[:, nkt + 1 * NJ:nkt + 2 * NJ]
            g2_sb = vec_sb[:, nkt + 2 * NJ:nkt + 3 * NJ]
            bb2_sb = vec_sb[:, nkt + 3 * NJ:nkt + 4 * NJ]
            b1_sb = vec_sb[:, nkt + 4 * NJ:nkt + 5 * NJ]
            b2_sb = vec_sb[:, nkt + 5 * NJ:nkt + 6 * NJ]
            bb1b2_sb = vec_sb[:, nkt + 6 * NJ:nkt + 7 * NJ]  # ln1_b + b2
            lnw = persist.tile([P, P], BF16)         # 1/D for LN stat matmuls
            eps_sb = persist.tile([P, 1], F32)
            # attention head outputs (x32 scale), feature-major; tile j rows
            # 0:64 = head 2j+1, rows 64:128 = head 2j (host permutes Wo rows).
            outT = persist.tile([P, NJ, NQ], F8)

            def load_inputs():
                """Big loads, spread across engine queues so descriptor
                generation runs in parallel; each DMA fully contiguous
                (strided DMAs pay ~2us of per-row descriptor overhead).
                SP queue: the V/K-proj critical path first, tail weights
                after (so they can't steal bandwidth from startup)."""
                wv_r = wv.rearrange("p (c k m) -> p c k m", c=2, k=NJ)
                nc.sync.dma_start(wv_sb[:, 0], wv_r[:, 0])
                yt_r = yt.rearrange("p (k m) -> p k m", k=NJ)
                nc.sync.dma_start(yt_sb[:, 0:4], yt_r[:, 0:4])
                nc.sync.dma_start(yt_sb[:, 4:NJ], yt_r[:, 4:NJ])
                nc.sync.dma_start(wk_sb, wk.rearrange("p (k m) -> p k m", k=NJ))
                nc.sync.dma_start(wv_sb[:, 1], wv_r[:, 1])
                nc.sync.dma_start(wo_sb, wo.rearrange("p (k m) -> p k m", k=NJ))
                nc.sync.dma_start(xt_sb, xt.rearrange("p (j q) -> p j q", j=NJ))
                nc.sync.dma_start(w1_sb, w1.rearrange("p (k m) -> p k m", k=NJ))
                nc.sync.dma_start(w2_sb, w2.rearrange("p (k m) -> p k m", k=NJ))
                # ACT queue in parallel: vecs (first exp) + the Q-proj path.
                nc.scalar.dma_start(vec_sb, vecs)
                nc.scalar.dma_start(
                    xt8_sb, xt8.rearrange("p (j q) -> p j q", j=NJ))
                nc.scalar.dma_start(
                    wq_sb, wq.rearrange("p (k m) -> p k m", k=NJ))
                nc.vector.memset(lnw, 1.0 / D)
                nc.vector.memset(eps_sb, EPS)

            if hoist:
                load_inputs()
            with (tc.For_i(0, reps) if reps > 1 else nullcontext()):
                if not hoist:
                    load_inputs()

            with tc.tile_pool(name="attn", bufs=1) as big:
                # V (x32) with an interleaved ones column per head
                v_sb = big.tile([P, nkt, H, HD + 1], F8)
                nc.vector.memset(v_sb[:, :, :, HD:HD + 1], 1.0)

                # ---- V = Y @ Wv.T x32 (DoubleRow over feature pairs),
                # N=512 so LDWEIGHTS amortizes over the full stream; halves
                # so the first matmul only waits for yt + half of Wv.
                # Evictions alternate DVE/ACT to halve the copy backlog. ----
                for ci in range(2):
                    for t in range(nkt):
                        ps = ps_tile("ps_v")
                        for kp in range(NP):
                            nc.tensor.matmul(
                                ps,
                                yt_sb[:, 2 * kp:2 * kp + 2, t * P:(t + 1) * P],
                                wv_sb[:, ci, 2 * kp:2 * kp + 2, :],
                                start=(kp == 0), stop=(kp == NP - 1),
                                perf_mode=DR,
                            )
                        dst = v_sb[:, t, ci * 8:(ci + 1) * 8, 0:HD]
                        if t % 2 == 0:
                            nc.vector.tensor_copy(dst, ps)
                        else:
                            nc.scalar.activation(dst, ps, AF.Copy)

                # ---- per head-pair: KT, QT, scoresT, exp, attnV ----
                with tc.tile_pool(name="qk", bufs=2) as qkp, \
                     tc.tile_pool(name="exp", bufs=4) as ep, \
                     tc.tile_pool(name="stage", bufs=3) as stp:
                    deferred = []   # (ps_e, ps_o, rc_e, rc_o, j) of prev pair

                    def flush_deferred():
                        for (pse, pso, rce, rco, jj) in deferred:
                            # replicate the reciprocal rows across 64
                            # partitions on the (idle) Pool engine.  HW ucode
                            # broadcasts the physical partition 0, so first
                            # hop the row from partition 64 to 0 via a tiny
                            # shift DMA.
                            rc0o = stp.tile([P, NQ], F32R, tag="rc0", name="rc0o")
                            nc.sync.dma_start(rc0o[0:1, :], rco[HD:HD + 1, :])
                            rc0e = stp.tile([P, NQ], F32R, tag="rc0", name="rc0e")
                            nc.sync.dma_start(rc0e[0:1, :], rce[HD:HD + 1, :])
                            rcb_o = stp.tile([P, NQ], F32R, tag="rcb", name="rcb_o")
                            nc.gpsimd.partition_broadcast(
                                rcb_o[0:HD, :], rc0o[0:1, :])
                            rcb_e = stp.tile([P, NQ], F32R, tag="rcb", name="rcb_e")
                            nc.gpsimd.partition_broadcast(
                                rcb_e[0:HD, :], rc0e[0:1, :])
                            # odd head -> outT rows 0:64 directly (x32 kept)
                            nc.vector.tensor_mul(
                                outT[0:HD, jj, :], pso[0:HD, :], rcb_o[0:HD, :])
                            # even head -> staging, partition-shift to 64:128
                            tmp = stp.tile([P, NQ], F8, tag="tmp", name="tmp")
                            nc.vector.tensor_mul(
                                tmp[0:HD, :], pse[0:HD, :], rcb_e[0:HD, :])
                            nc.sync.dma_start(outT[HD:P, jj, :], tmp[0:HD, :])
                        deferred.clear()

                    def emit_kq(j):
                        """K^T + Q^T matmuls for pair j -> (kt_j, qt_j), both
                        x32-scaled fp8.  Streams only nkp keys; pad region
                        zeroed on the first ring pass."""
                        ms = slice(j * P, (j + 1) * P)
                        kt_j = qkp.tile([P, nt], F8, tag="kt", name="kt_j")
                        if nkp < nt:
                            nc.vector.memset(kt_j[:, nkp:nt], 0.0)
                        # DoubleRow chunks of 512 keys (N=512 amortizes the
                        # LDWEIGHTS), then a <=480 tail (normal mode below
                        # 128 free; DR above).
                        chunks = []
                        k0 = 0
                        while k0 < nkp:
                            kn = min(512, nkp - k0)
                            chunks.append((k0, kn))
                            k0 += kn
                        for (k0, kn) in chunks:
                            ks = slice(k0, k0 + kn)
                            ps = ps_tile("ps_k")
                            if kn >= P:
                                for kp in range(NP):
                                    nc.tensor.matmul(
                                        ps[:, 0:kn],
                                        wk_sb[:, 2 * kp:2 * kp + 2, ms],
                                        yt_sb[:, 2 * kp:2 * kp + 2, ks],
                                        start=(kp == 0), stop=(kp == NP - 1),
                                        perf_mode=DR,
                                    )
                            else:
                                for k in range(NJ):
                                    nc.tensor.matmul(
                                        ps[:, 0:kn], wk_sb[:, k, ms],
                                        yt_sb[:, k, ks],
                                        start=(k == 0), stop=(k == NJ - 1),
                                    )
                            nc.vector.tensor_copy(kt_j[:, ks], ps[:, 0:kn])
                        # Q^T m-tile j (x32; /8 folded into exp scale):
                        # one 4-instruction DR group over the full 512.
                        qt_j = qkp.tile([P, NQ], F8, tag="qt", name="qt_j")
                        ps = ps_tile("ps_q")
                        for kp in range(NP):
                            nc.tensor.matmul(
                                ps,
                                wq_sb[:, 2 * kp:2 * kp + 2, ms],
                                xt8_sb[:, 2 * kp:2 * kp + 2, :],
                                start=(kp == 0), stop=(kp == NP - 1),
                                perf_mode=DR,
                            )
                        nc.vector.tensor_copy(qt_j, ps)
                        return kt_j, qt_j

                    def emit_scores(j, kt_j, qt_j):
                        """scoresT + exp for pair j, row-packed on the PE.
                        psum = 8192*s; exp applies scale=1/8192 + mask bias."""
                        exp_e = ep.tile([P, nkt, NQ], F8, tag="exp", name="exp_e")
                        exp_o = ep.tile([P, nkt, NQ], F8, tag="exp", name="exp_o")
                        for t in range(nkt):
                            ks = slice(t * P, (t + 1) * P)
                            ps0 = ps_tile("ps_s0")
                            nc.tensor.matmul(
                                ps0, kt_j[0:HD, ks], qt_j[0:HD, :],
                                start=True, stop=True, tile_position=(0, 0),
                            )
                            ps1 = ps_tile("ps_s1")
                            nc.tensor.matmul(
                                ps1, kt_j[HD:P, ks], qt_j[HD:P, :],
                                start=True, stop=True, tile_position=(HD, 0),
                            )
                            nc.scalar.activation(
                                exp_e[:, t, :], ps0, AF.Exp,
                                bias=mask_sb[:, t:t + 1], scale=ESC)
                            nc.scalar.activation(
                                exp_o[:, t, :], ps1, AF.Exp,
                                bias=mask_sb[:, t:t + 1], scale=ESC)
                        return exp_e, exp_o

                    def emit_attnv(j, exp_e, exp_o):
                        """attnV: lhsT = [V_h*32 | ones] (M=65) -> rows 0:64
                        x32 numerator, row 64 = softmax denominator."""
                        ps_e = ps_tile("ps_ae")
                        ps_o = ps_tile("ps_ao")
                        for t in range(nkt):
                            st, sp = t == 0, t == nkt - 1
                            nc.tensor.matmul(
                                ps_e[0:HD + 1, :], v_sb[:, t, 2 * j, :],
                                exp_e[:, t, :], start=st, stop=sp,
                            )
                        for t in range(nkt):
                            st, sp = t == 0, t == nkt - 1
                            nc.tensor.matmul(
                                ps_o[0:HD + 1, :], v_sb[:, t, 2 * j + 1, :],
                                exp_o[:, t, :], start=st, stop=sp,
                            )
                        # reciprocals (partition 64, f32r) on DVE now; divide
                        # + broadcast run one pair later via flush_deferred
                        rc_e = stp.tile([P, NQ], F32R, tag="rc", name="rc_e")
                        nc.vector.reciprocal(
                            rc_e[HD:HD + 1, :], ps_e[HD:HD + 1, :])
                        rc_o = stp.tile([P, NQ], F32R, tag="rc", name="rc_o")
                        nc.vector.reciprocal(
                            rc_o[HD:HD + 1, :], ps_o[HD:HD + 1, :])
                        deferred.append((ps_e, ps_o, rc_e, rc_o, j))

                    # software pipeline: next pair's K/Q matmuls are issued
                    # between this pair's scores and attnV so PE never waits
                    # on the DVE evictions or the exp activations.
                    kq = emit_kq(0)
                    for j in range(NPAIR):
                        flush_deferred()
                        exps = emit_scores(j, *kq)
                        if j + 1 < NPAIR:
                            kq = emit_kq(j + 1)
                        emit_attnv(j, *exps)
                    flush_deferred()

            # ---- tail: O-proj + LN1 + FFN + LN2, 2 query-chunks ----
            with tc.tile_pool(name="tail", bufs=1) as tl, \
                 tc.tile_pool(name="ln", bufs=8) as lnp:

                x1 = tl.tile([P, NJ, NQ], BF16)
                xsq = tl.tile([P, NJ, NQ], BF16)
                hT = tl.tile([P, NJ, NQ], BF16)
                hTb2 = tl.tile([P, NJ, NQ], BF16)
                ff1 = tl.tile([P, NJ, NQ], F8)
                x2 = tl.tile([P, NJ, NQ], BF16)
                o_sb = tl.tile([P, NQ, NJ], F32)   # query-major for the store

                def oproj(c):
                    """Z = outT.T @ Wo.T via DoubleRow over feature pairs;
                    psum = 512*Z, evicted with a 1/512 scale + X residual."""
                    qs = slice(c * QC, (c + 1) * QC)
                    for m in range(NJ):
                        ps = ps_tile("ps_z")
                        for gp in range(NP):
                            nc.tensor.matmul(
                                ps[:, 0:QC],
                                wo_sb[:, 2 * gp:2 * gp + 2, m * P:(m + 1) * P],
                                outT[:, 2 * gp:2 * gp + 2, qs],
                                start=(gp == 0), stop=(gp == NP - 1),
                                perf_mode=DR,
                            )
                        nc.vector.scalar_tensor_tensor(
                            x1[:, m, qs], ps[:, 0:QC], OSC, xt_sb[:, m, qs],
                            op0=mybir.AluOpType.mult, op1=mybir.AluOpType.add)
                        # eager square so LN1 stats don't wait on ACT later
                        nc.scalar.activation(
                            xsq[:, m, qs], x1[:, m, qs], AF.Square)

                def ln_stats(x_sb, qs, name):
                    """mean/E[x^2] matmuls for query-slice qs (squares
                    already in xsq, computed eagerly at eviction)."""
                    qn = qs.stop - qs.start
                    ps_m = ps_tile(name + "_m")
                    for jj in range(NJ):
                        nc.tensor.matmul(
                            ps_m[:, 0:qn], lnw, x_sb[:, jj, qs],
                            start=(jj == 0), stop=(jj == NJ - 1))
                    ps_v = ps_tile(name + "_v")
                    for jj in range(NJ):
                        nc.tensor.matmul(
                            ps_v[:, 0:qn], lnw, xsq[:, jj, qs],
                            start=(jj == 0), stop=(jj == NJ - 1))
                    return ps_m, ps_v

                # output is query-major [p, q, j] so each wave's store is one
                # fully-contiguous DMA (strided stores pay the ~2.3us
                # 128-descriptor floor per call)
                outt_r = outt.rearrange("p (q j) -> p q j", j=NJ)

                def ln_norm(x_sb, ps_m, ps_v, qs, gv, bv, dest, store=False,
                            dest2=None, bv2=None):
                    """DVE/Pool chain: normalize query-slice qs into dest.
                    Stats psums are read in place (no mean eviction); rstd
                    comes from one ACT Rsqrt.  dest2/bv2 emit a second
                    biased copy (hT + b2 for the ffn2 eviction) on Pool."""
                    qn = qs.stop - qs.start
                    mean = lnp.tile([P, QC], F32, tag="lnt", name="mean")[:, 0:qn]
                    nc.vector.tensor_copy(mean, ps_m[:, 0:qn])
                    var = lnp.tile([P, QC], F32, tag="lnt", name="var")[:, 0:qn]
                    nc.vector.tensor_mul(var, mean, mean)
                    nc.vector.tensor_tensor(
                        var, ps_v[:, 0:qn], var, mybir.AluOpType.subtract)
                    sd = lnp.tile([P, QC], F32, tag="lnt", name="sd")[:, 0:qn]
                    nc.scalar.activation(sd, var, AF.Sqrt, bias=eps_sb, scale=1.0)
                    rstd = lnp.tile([P, QC], BF16, tag="lnr", name="rstd")[:, 0:qn]
                    nc.vector.reciprocal(rstd, sd)
                    mrs = lnp.tile([P, QC], BF16, tag="lnr", name="mrs")[:, 0:qn]
                    nc.vector.tensor_mul(mrs, mean, rstd)
                    for jj in range(NJ):
                        t = lnp.tile([P, QC], BF16, tag="lnb", name="t")[:, 0:qn]
                        nc.vector.tensor_mul(t, x_sb[:, jj, qs], rstd)
                        nc.vector.tensor_tensor(
                            t, t, mrs, mybir.AluOpType.subtract)
                        # g*t + b on the (tail-idle) ACT engine, pipelined
                        # with the next tile's DVE ops; the store variant
                        # writes query-major so the DMA is contiguous
                        d = dest[:, qs, jj] if store else dest[:, jj, qs]
                        nc.scalar.activation(
                            d, t, AF.Identity,
                            bias=bv[:, jj:jj + 1], scale=gv[:, jj:jj + 1])
                        if dest2 is not None:
                            nc.scalar.activation(
                                dest2[:, jj, qs], t, AF.Identity,
                                bias=bv2[:, jj:jj + 1], scale=gv[:, jj:jj + 1])
                    if store:
                        nc.sync.dma_start(
                            outt_r[:, qs, :], dest[:, qs, :])

                def ffn1(c):
                    """ff1 = relu(hT @ W1.T + b1) * 8, evicted fp8 (b1
                    host-premultiplied by 8)."""
                    qs = slice(c * QC, (c + 1) * QC)
                    for m in range(NJ):
                        ps = ps_tile("ps_f1")
                        for k in range(NJ):
                            nc.tensor.matmul(
                                ps[:, 0:QC], w1_sb[:, k, m * P:(m + 1) * P],
                                hT[:, k, qs],
                                start=(k == 0), stop=(k == NJ - 1))
                        nc.scalar.activation(
                            ff1[:, m, qs], ps[:, 0:QC], AF.Relu,
                            bias=b1_sb[:, m:m + 1], scale=F1S)

                def ffn2(qs):
                    """x2 = 32*ff psum / 32 + (hT + b2) in one DVE op
                    (hTb2 pre-biased during LN1); eager square on ACT.
                    DoubleRow over ff-feature pairs."""
                    qn = qs.stop - qs.start
                    for m in range(NJ):
                        ps = ps_tile("ps_f2")
                        for kp in range(NP):
                            nc.tensor.matmul(
                                ps[:, 0:qn],
                                w2_sb[:, 2 * kp:2 * kp + 2, m * P:(m + 1) * P],
                                ff1[:, 2 * kp:2 * kp + 2, qs],
                                start=(kp == 0), stop=(kp == NP - 1),
                                perf_mode=DR,
                            )
                        nc.vector.scalar_tensor_tensor(
                            x2[:, m, qs], ps[:, 0:qn], 1.0 / (F1S * W2S),
                            hTb2[:, m, qs],
                            op0=mybir.AluOpType.mult, op1=mybir.AluOpType.add)
                        nc.scalar.activation(
                            xsq[:, m, qs], x2[:, m, qs], AF.Square)

                # software pipeline over the 2 chunks: each LN's DVE chain is
                # issued right after its stats so no engine-queue inversion,
                # and overlaps the next PE stage.  The second half of ffn2 +
                # LN2 runs in shrinking waves (128/64/64) so the serial
                # end-of-kernel chain is as short as possible.
                c0, c1 = slice(0, QC), slice(QC, NQ)
                oproj(0)
                s1m0, s1v0 = ln_stats(x1, c0, "ln1c0")
                ln_norm(x1, s1m0, s1v0, c0, g1_sb, bb1_sb, hT,
                        dest2=hTb2, bv2=bb1b2_sb)
                oproj(1)
                s1m1, s1v1 = ln_stats(x1, c1, "ln1c1")
                ln_norm(x1, s1m1, s1v1, c1, g1_sb, bb1_sb, hT,
                        dest2=hTb2, bv2=bb1b2_sb)
                ffn1(0)
                ffn2(c0)
                ffn1(1)
                s2m0, s2v0 = ln_stats(x2, c0, "ln2c0")
                ln_norm(x2, s2m0, s2v0, c0, g2_sb, bb2_sb, o_sb, store=True)
                for sq in (slice(256, 384), slice(384, 448), slice(448, 512)):
                    ffn2(sq)
                    sm, sv = ln_stats(x2, sq, f"ln2w{sq.start}")
                    ln_norm(x2, sm, sv, sq, g2_sb, bb2_sb, o_sb, store=True)

    nc.compile()
    return nc


_NC_CACHE: dict = {}


def _get_nc(nkt: int, nkp: int | None = None) -> bass.Bass:
    key = (nkt, nkp)
    if key not in _NC_CACHE:
        _NC_CACHE[key] = build_nc(nkt, nkp=nkp)
    return _NC_CACHE[key]


def _bf16(a) -> np.ndarray:
    return np.ascontiguousarray(np.asarray(a, np.float32)).astype(
        ml_dtypes.bfloat16)


def _fp8(a, scale=1.0) -> np.ndarray:
    """TRN e4m3 quantize (clip +-240) with host-side scale."""
    v = np.clip(np.asarray(a, np.float32) * np.float32(scale), -240.0, 240.0)
    return np.ascontiguousarray(v).astype(ml_dtypes.float8_e4m3fn)


def _arrange_w(wt: np.ndarray) -> np.ndarray:
    """[D, D] (in-feat, out-feat) -> [128, NJ*D] with [p, k, m] layout."""
    return np.ascontiguousarray(
        wt.reshape(NJ, P, D).transpose(1, 0, 2).reshape(P, NJ * D))


def _prep_inputs(X, Y, mask_y, Wq, Wk, Wv, Wo, ln1_g, ln1_b, ln2_g, ln2_b,
                 W1, b1, W2, b2):
    X = np.asarray(X, np.float32)
    Y = np.asarray(Y, np.float32)
    mask_y = np.asarray(mask_y)
    B = X.shape[0]

    counts = [int(mask_y[b].sum()) for b in range(B)]
    nkt = 5 if max(counts) <= 5 * P else (max(counts) + P - 1) // P
    nt = nkt * P
    nkp = min(nt, max(32, -(-max(counts) // 32) * 32))

    # transposed weights (torch Linear: x @ W.T -> lhsT rows = W.T);
    # attention weights x32 (x16 for Wo) in fp8, FFN weights bf16.
    wqt = _fp8(np.asarray(Wq, np.float32).T, WS)
    wkt = _fp8(np.asarray(Wk, np.float32).T, WS)
    wvt = _fp8(np.asarray(Wv, np.float32).T, WS)
    w1t = _bf16(np.asarray(W1, np.float32).T)
    w2t = _fp8(np.asarray(W2, np.float32).T, W2S)
    # outT tile j holds head 2j+1 in rows 0:64, head 2j in rows 64:128
    perm = np.empty(D, dtype=np.int64)
    for j in range(NJ):
        perm[j * P:j * P + HD] = (2 * j + 1) * HD + np.arange(HD)
        perm[j * P + HD:(j + 1) * P] = (2 * j) * HD + np.arange(HD)
    wot = _fp8(np.asarray(Wo, np.float32).T[perm], WS / 2.0)

    vec = lambda v: np.asarray(v, np.float32).reshape(NJ, P).T
    b1s = np.asarray(b1, np.float32) * np.float32(F1S)  # ff1 evicts at x8
    bb1b2 = np.asarray(ln1_b, np.float32) + np.asarray(b2, np.float32)
    vtail = np.concatenate(
        [vec(v) for v in (ln1_g, ln1_b, ln2_g, ln2_b, b1s, b2, bb1b2)], axis=1)
    # wv rearranged into [p, ci, k, m%512] halves for contiguous DMAs
    wv_arr = _arrange_w(wvt).reshape(P, NJ, 2, 512).transpose(0, 2, 1, 3)
    shared = dict(
        wq=_arrange_w(wqt), wk=_arrange_w(wkt),
        wv=np.ascontiguousarray(wv_arr.reshape(P, NJ * D)),
        wo=_arrange_w(wot), w1=_arrange_w(w1t), w2=_arrange_w(w2t),
    )

    per_batch = {}
    for b in range(B):
        idx = np.flatnonzero(mask_y[b])
        nv = len(idx)
        Yc = np.zeros((nt, D), np.float32)
        bias = np.full(nt, NEG, np.float32)
        if nv == 0:
            bias[0] = 0.0   # zero sentinel key -> attn out = 0/1 = 0
        else:
            Yc[:nv] = Y[b][idx]
            bias[:nv] = 0.0
        # yt layout [p, k, key]: Yc^T[k*128+p, key]
        ytc = _fp8(Yc.T).reshape(NJ, P, nt).transpose(1, 0, 2)
        per_batch[b] = (
            np.ascontiguousarray(ytc.reshape(P, NJ * nt)),
            np.ascontiguousarray(
                np.concatenate([bias.reshape(nkt, P).T, vtail], axis=1)),
        )

    in_maps = []
    for core in range(8):
        b, half = divmod(core, 2)
        q0 = half * NQ
        m = dict(shared)
        # xt layout [p, j, q]: X^T[j*128+p, q]
        xs = X[b, q0:q0 + NQ, :].T
        m["xt"] = np.ascontiguousarray(
            _bf16(xs).reshape(NJ, P, NQ).transpose(1, 0, 2).reshape(P, NJ * NQ))
        m["xt8"] = np.ascontiguousarray(
            _fp8(xs).reshape(NJ, P, NQ).transpose(1, 0, 2).reshape(P, NJ * NQ))
        m["yt"], m["vecs"] = per_batch[b]
        in_maps.append(m)
    return in_maps, (nkt, nkp)


def unpack_output(arrs) -> np.ndarray:
    """arrs: per-core [128, NQ*NJ] f32 (query-major) -> [4, 1024, D]."""
    out = np.empty((4, 1024, D), dtype=np.float32)
    for core in range(8):
        b, half = divmod(core, 2)
        q0 = half * NQ
        a = np.asarray(arrs[core]).reshape(P, NQ, NJ)
        out[b, q0:q0 + NQ, :] = a.transpose(1, 2, 0).reshape(NQ, D)
    return out


def kernel(**inputs) -> np.ndarray:
    in_maps, (nkt, nkp) = _prep_inputs(**inputs)
    res = run_bass_kernel_spmd(_get_nc(nkt, nkp), in_maps,
                               core_ids=list(range(8)))
    return unpack_output([res.results[c]["outt"] for c in range(8)])
